# revision 1
# baseline (speedup 1.0000x reference)
"""BlipAttention kernel for 8 Trainium2 NeuronCores.

Strategy: data-parallel over batch (16 batches -> 2 per core), no collectives.
Per core: fused QKV projection + 16-head scaled-dot-product attention + output
projection on the PE, bf16 matmuls with fp32 PSUM accumulation.

v2 restructure (from trace analysis of the v1 kernel):
  - x is uploaded bf16 and loaded feature-major via XBAR transposing DMAs
    (no PE transposes, no PSUM->SBUF copies for x^T).
  - q/k projection accumulates into a 2-bank [128,578] PSUM tile; one wide
    DVE tensor_scalar_add applies the per-feature bias and casts to bf16.
  - v projection reuses its stationary x^T tile across two 440-col chunks
    per 2-bank PSUM tile; strided DVE copies regroup 88-col heads into the
    97-wide vsb groups (cols 88..96 stay 1.0 for the fused denominator).
  - v-bias is folded into the projection bias on host (softmax rows sum to
    one), removing the rank-1 bias matmuls.
  - scores^T per (head, k-tile) land in one 2-bank PSUM tile; exp is a
    single 578-wide ACT instruction (ACT runs nothing but exp).
  - softmax normalization: DVE reciprocal of the fused denominator row,
    gpsimd partition_broadcast, one DVE mul straight out of PSUM.
  - output projection is computed feature-major (stationary w_proj tile,
    moving apk), so its bias is per-partition and fused into the DVE
    PSUM->SBUF copy; out^T is written bf16 to DRAM and transposed on host.
  - per-batch buffers + split PSUM tags (sc/pv/acc) + interleaved emission
    let the scheduler overlap batch b+1 projections with batch b attention,
    keeping the PE dense and HAM-warm.
"""

import contextlib
import os

import numpy as np
import ml_dtypes

DBG = os.environ.get("BLIP_DBG") == "1"

import concourse.bass as bass
import concourse.tile as tile
from concourse import bacc, mybir
from concourse.bass_utils import run_bass_kernel_spmd

F32 = mybir.dt.float32
F32R = mybir.dt.float32r
BF16 = mybir.dt.bfloat16

N_CORES = 8
B_TOTAL, S, D = 16, 577, 1408
H, HD = 16, 88
SCALE = HD ** -0.5
B = B_TOTAL // N_CORES          # batches per core = 2
T = B * S                       # tokens per core = 1154
SP = S + 1                      # padded q-span = 578
SPAD = 592                      # host-padded token rows per batch (37*16)
KT = D // 128                   # 11 k-tiles over D
VG = 97                         # v group width per head: 88 v cols + 9 ones
DEN = 96                        # psum partition of the softmax denominator

# chunks of a 578-wide span, each within one PSUM bank
CH_S = [(0, 512), (512, 66)]
# v projection chunk-groups: (src col base, src width, subs) where each sub
# is (moving col, psum col, width, first head); subs are head-aligned
# (multiple of 88) and live in one PSUM bank of the 2-bank acc tile
VCH = [
    (0, 880, [(0, 0, 440, 0), (440, 512, 440, 5)]),
    (880, 528, [(0, 0, 440, 10), (440, 512, 88, 15)]),
]


def _tok_tiles():
    out = []
    for tt in range((S + 127) // 128):
        t0 = tt * 128
        out.append((tt, t0, min(128, S - t0)))
    return out


TT = len(_tok_tiles())          # 5


def build_program():
    nc = bacc.Bacc("TRN2", target_bir_lowering=False, debug=False,
                   num_devices=N_CORES)

    x_ap = nc.dram_tensor("xT_bf", [B, D, SPAD], BF16, kind="ExternalInput").ap()
    wqk_ap = nc.dram_tensor("wqk_bf", [D, 2 * D], BF16, kind="ExternalInput").ap()
    wv_ap = nc.dram_tensor("wv_bf", [D, D], BF16, kind="ExternalInput").ap()
    wp_ap = nc.dram_tensor("wp_bf", [D, D], BF16, kind="ExternalInput").ap()
    bqk_ap = nc.dram_tensor("b_qk_col", [2 * D, 1], F32, kind="ExternalInput").ap()
    beff_ap = nc.dram_tensor("b_eff_col", [D, 1], F32, kind="ExternalInput").ap()
    outT_ap = nc.dram_tensor("outT", [D, T], BF16, kind="ExternalOutput").ap()
    if DBG:
        dbg_xT = nc.dram_tensor("dbg_xT", [128, SPAD], BF16,
                                kind="ExternalOutput").ap()
        dbg_q = nc.dram_tensor("dbg_q", [HD, SP], BF16,
                               kind="ExternalOutput").ap()
        dbg_k = nc.dram_tensor("dbg_k", [HD, SP], BF16,
                               kind="ExternalOutput").ap()
        dbg_v = nc.dram_tensor("dbg_v", [128, H * VG], BF16,
                               kind="ExternalOutput").ap()
        dbg_e = nc.dram_tensor("dbg_e", [128, SP], BF16,
                               kind="ExternalOutput").ap()
        dbg_at = nc.dram_tensor("dbg_at", [HD, SP], BF16,
                                kind="ExternalOutput").ap()
        dbg_apk = nc.dram_tensor("dbg_apk", [128, SP], BF16,
                                 kind="ExternalOutput").ap()

    with tile.TileContext(nc) as tc, contextlib.ExitStack() as ctx:
        p_xT = ctx.enter_context(tc.tile_pool(name="xT", bufs=2 * KT))
        p_vsb = ctx.enter_context(tc.tile_pool(name="vsb", bufs=2 * TT))
        p_qk = ctx.enter_context(tc.tile_pool(name="qk", bufs=36))
        p_expT = ctx.enter_context(tc.tile_pool(name="expT", bufs=8))
        p_at = ctx.enter_context(tc.tile_pool(name="at", bufs=4))
        p_apk = ctx.enter_context(tc.tile_pool(name="apk", bufs=2 * KT))
        p_qksb = ctx.enter_context(tc.tile_pool(name="qksb", bufs=4))
        p_rec = ctx.enter_context(tc.tile_pool(name="rec", bufs=2))
        p_recb = ctx.enter_context(tc.tile_pool(name="recb", bufs=2))
        p_wqk = ctx.enter_context(tc.tile_pool(name="wqk", bufs=4))
        p_wp = ctx.enter_context(tc.tile_pool(name="wp", bufs=4))
        p_wv = ctx.enter_context(tc.tile_pool(name="wv", bufs=12))
        p_bias = ctx.enter_context(tc.tile_pool(name="bias", bufs=6))
        p_ot = ctx.enter_context(tc.tile_pool(name="ot", bufs=4))

        p_sc = ctx.enter_context(tc.tile_pool(name="psc", bufs=2, space="PSUM"))
        p_pv = ctx.enter_context(tc.tile_pool(name="ppv", bufs=2, space="PSUM"))
        p_acc = ctx.enter_context(tc.tile_pool(name="pacc", bufs=2, space="PSUM"))

        # x^T tiles, per batch, loaded via XBAR transposing DMA
        xT = [[p_xT.tile([128, SPAD], BF16, tag="xT", name=f"xT{b}_{k}")
               for k in range(KT)] for b in range(B)]
        # v tiles per batch; memset to 1.0 once, v-copies only touch cols 0..87
        # of each 97-group so cols 88..96 stay 1.0 (fused softmax denominator)
        vsb = [[p_vsb.tile([128, H * VG], BF16, tag="vsb", name=f"vsb{b}_{tt}")
                for tt in range(TT)] for b in range(B)]
        apk = [[p_apk.tile([128, SP], BF16, tag="apk", name=f"apk{b}_{k}")
                for k in range(KT)] for b in range(B)]
        qh = [[None] * H for _ in range(B)]
        kh = [[None] * H for _ in range(B)]

        def emit_warmup():
            # ~3.5us of back-to-back dummy matmuls so the HAM clock gate is
            # released (K=8/8) by the time real operands arrive
            wsrc = p_qksb.tile([128, SP], BF16, tag="qksb", name="warm_src")
            nc.gpsimd.memset(wsrc[:], 0.0)
            wps = p_acc.tile([128, 512], F32, tag="acc", name="warm_ps")
            for _ in range(64):
                nc.tensor.matmul(wps[:, 0:128], wsrc[:, 0:128],
                                 wsrc[:, 0:128], start=True, stop=True)

        def emit_loads(b):
            for k in range(KT):
                eng = nc.sync if k % 2 == 0 else nc.scalar
                eng.dma_start(xT[b][k][:],
                              x_ap[b, k * 128:(k + 1) * 128, :])
            for tt in range(TT):
                nc.gpsimd.memset(vsb[b][tt][:], 1.0)

        def emit_vproj(b):
            """generator: one unit per (chunk-group, token-tile)"""
            for cg, (s0, sw, subs) in enumerate(VCH):
                wvt = []
                for k in range(KT):
                    wv_t = p_wv.tile([128, 880], BF16, tag="wv",
                                     name=f"wv{b}_{cg}_{k}")
                    nc.gpsimd.dma_start(
                        wv_t[:, 0:sw],
                        wv_ap[k * 128:(k + 1) * 128, s0:s0 + sw])
                    wvt.append(wv_t)
                for tt, t0, ts in _tok_tiles():
                    acc = p_acc.tile([128, 952], F32, tag="acc", name="vacc")
                    for ki in range(KT):
                        for (m0, p0, w, h0) in subs:
                            nc.tensor.matmul(
                                acc[0:ts, p0:p0 + w],
                                xT[b][ki][:, t0:t0 + ts],
                                wvt[ki][:, m0:m0 + w],
                                start=(ki == 0), stop=(ki == KT - 1))
                    for (m0, p0, w, h0) in subs:
                        nh = w // HD
                        dst = vsb[b][tt][0:ts, :].rearrange(
                            "p (h g) -> p h g", g=VG)[:, h0:h0 + nh, 0:HD]
                        src = acc[0:ts, p0:p0 + w].rearrange(
                            "p (h g) -> p h g", g=HD)
                        nc.vector.tensor_copy(dst, src)
                    yield

        def _ship_heads(b, which, fidx, qksb):
            dst_l = qh[b] if which == 0 else kh[b]
            f_lo, f_hi = fidx * 128, fidx * 128 + 128
            for h in range(f_lo // HD, min(H, (f_hi + HD - 1) // HD)):
                s0 = max(f_lo, h * HD)
                s1 = min(f_hi, (h + 1) * HD)
                if s1 <= s0:
                    continue
                if dst_l[h] is None:
                    dst_l[h] = p_qk.tile([HD, SP], BF16, tag="qk",
                                         name=f"qk{b}_{which}_{h}")
                r0 = s0 - h * HD
                nc.scalar.dma_start(dst_l[h][r0: r0 + (s1 - s0), :],
                                    qksb[s0 - f_lo: s1 - f_lo, :])

        def emit_qkproj(b):
            """generator: one unit per m-tile (22 of them)"""
            for m in range(2 * KT):
                col = m * 128
                wqt = p_wqk.tile([128, KT * 128], BF16, tag="wqk",
                                 name=f"wq{b}_{m}")
                nc.sync.dma_start(
                    wqt[:].rearrange("p (k c) -> p k c", k=KT),
                    wqk_ap[:, col: col + 128].rearrange("(k p) c -> p k c",
                                                        p=128))
                bq = p_bias.tile([128, 1], F32, tag="bias", name="bq")
                nc.sync.dma_start(bq[:], bqk_ap[col: col + 128, :])
                acc = p_acc.tile([128, SP], F32, tag="acc", name="qkacc")
                for ki in range(KT):
                    for (lc, w) in CH_S:
                        nc.tensor.matmul(acc[0:128, lc:lc + w],
                                         wqt[:, ki * 128:(ki + 1) * 128],
                                         xT[b][ki][:, lc:lc + w],
                                         start=(ki == 0), stop=(ki == KT - 1))
                qksb = p_qksb.tile([128, SP], BF16, tag="qksb")
                nc.vector.tensor_scalar_add(qksb[:], acc[0:128, 0:SP], bq[:])
                which, fidx = (0, m) if m < KT else (1, m - KT)
                _ship_heads(b, which, fidx, qksb)
                yield

        def emit_attention(b):
            """generator: one unit per head; scores/exp/PV run in 1-bank
            q-strips so two strips pipeline through the 2-buf sc tag"""
            for h in range(H):
                pv = [p_pv.tile([VG, 512], F32, tag="pv", name=f"pv{si}")
                      for si in range(2)]
                et00 = None
                for tt, t0, ts in _tok_tiles():
                    for si, (lc, w) in enumerate(CH_S):
                        sc = p_sc.tile([128, 512], F32, tag="sc", name="sc")
                        nc.tensor.matmul(sc[0:ts, 0:w],
                                         kh[b][h][:, t0:t0 + ts],
                                         qh[b][h][:, lc:lc + w],
                                         start=True, stop=True)
                        et = p_expT.tile([128, 512], BF16, tag="expT")
                        nc.scalar.activation(et[0:ts, 0:w], sc[0:ts, 0:w],
                                             mybir.ActivationFunctionType.Exp,
                                             scale=SCALE)
                        if tt == 0 and si == 0:
                            et00 = et
                        nc.tensor.matmul(pv[si][0:VG, 0:w],
                                         vsb[b][tt][0:ts, h * VG:(h + 1) * VG],
                                         et[0:ts, 0:w],
                                         start=(tt == 0), stop=(tt == TT - 1))
                rec = p_rec.tile([1, SP], F32R, tag="rec", name="rec")
                with nc.allow_low_precision(reason="softmax reciprocal"):
                    nc.vector.reciprocal(rec[:, 0:512],
                                         pv[0][DEN:DEN + 1, 0:512])
                    nc.vector.reciprocal(rec[:, 512:SP],
                                         pv[1][DEN:DEN + 1, 0:66])
                recb = p_recb.tile([HD, SP], F32R, tag="recb", name="recb")
                nc.gpsimd.partition_broadcast(recb[:, 0:SP], rec[0:1, 0:SP])
                at = p_at.tile([HD, SP], BF16, tag="at", name="at")
                nc.vector.tensor_mul(at[:, 0:512], pv[0][0:HD, 0:512],
                                     recb[:, 0:512])
                nc.vector.tensor_mul(at[:, 512:SP], pv[1][0:HD, 0:66],
                                     recb[:, 512:SP])
                if DBG and b == 0 and h == 0:
                    nc.sync.dma_start(dbg_e[:, 0:512], et00[:, 0:512])
                    nc.sync.dma_start(dbg_at[:], at[:])
                f0 = h * HD
                k0, r0 = f0 // 128, f0 % 128
                n0 = min(HD, 128 - r0)
                nc.scalar.dma_start(apk[b][k0][r0: r0 + n0, :], at[0:n0, :])
                if n0 < HD:
                    nc.scalar.dma_start(apk[b][k0 + 1][0: HD - n0, :],
                                        at[n0:HD, :])
                yield

        def emit_outproj(b):
            """generator: one unit per output-feature tile (11 of them)"""
            for oc in range(KT):
                wpt = p_wp.tile([128, KT * 128], BF16, tag="wp",
                                name=f"wp{b}_{oc}")
                nc.sync.dma_start(
                    wpt[:].rearrange("p (k c) -> p k c", k=KT),
                    wp_ap[:, oc * 128:(oc + 1) * 128].rearrange(
                        "(k p) c -> p k c", p=128))
                be = p_bias.tile([128, 1], F32, tag="bias", name="be")
                nc.sync.dma_start(be[:], beff_ap[oc * 128:(oc + 1) * 128, :])
                acc = p_acc.tile([128, SP], F32, tag="acc", name="oacc")
                for ki in range(KT):
                    for (lc, w) in CH_S:
                        nc.tensor.matmul(acc[0:128, lc:lc + w],
                                         wpt[:, ki * 128:(ki + 1) * 128],
                                         apk[b][ki][:, lc:lc + w],
                                         start=(ki == 0), stop=(ki == KT - 1))
                ot = p_ot.tile([128, SP], BF16, tag="ot")
                nc.vector.tensor_scalar_add(ot[:], acc[0:128, 0:SP], be[:])
                nc.sync.dma_start(outT_ap[oc * 128:(oc + 1) * 128,
                                          b * S:(b + 1) * S],
                                  ot[:, 0:S])
                yield

        def drain(gen, n=None):
            done = 0
            for _ in gen:
                done += 1
                if n is not None and done >= n:
                    return False
            return True

        # ---- emission schedule ----
        emit_warmup()
        for b in range(B):
            emit_loads(b)

        drain(emit_vproj(0))
        drain(emit_qkproj(0))
        if DBG:
            nc.sync.dma_start(dbg_xT[:], xT[0][0][:])
            nc.sync.dma_start(dbg_q[:], qh[0][0][:])
            nc.sync.dma_start(dbg_k[:], kh[0][0][:])
            nc.sync.dma_start(dbg_v[:], vsb[0][0][:])

        # batch-0 attention interleaved with batch-1 projections
        att0 = emit_attention(0)
        vp1 = emit_vproj(1)
        qk1 = emit_qkproj(1)

        def next_unit_b1():
            for g in (vp1, qk1):
                try:
                    next(g)
                    return True
                except StopIteration:
                    continue
            return False

        for h in range(H):
            next(att0)
            next_unit_b1()
            next_unit_b1()
        while next_unit_b1():
            pass

        if DBG:
            nc.sync.dma_start(dbg_apk[:], apk[0][0][:])

        # batch-1 attention interleaved with batch-0 output projection
        att1 = emit_attention(1)
        op0 = emit_outproj(0)
        op0_left = KT
        for h in range(H):
            next(att1)
            if op0_left > 0:
                try:
                    next(op0)
                    op0_left -= 1
                except StopIteration:
                    op0_left = 0
        while op0_left > 0:
            try:
                next(op0)
                op0_left -= 1
            except StopIteration:
                break

        drain(emit_outproj(1))

    nc.compile()
    return nc


_NC_CACHE = None


def _get_nc():
    global _NC_CACHE
    if _NC_CACHE is None:
        _NC_CACHE = build_program()
    return _NC_CACHE


def make_in_maps(hidden_states, w_qkv, b_qkv, w_proj, b_proj):
    hidden_states = np.asarray(hidden_states, dtype=np.float32)
    w_qkv = np.ascontiguousarray(np.asarray(w_qkv, dtype=np.float32))
    b_qkv = np.asarray(b_qkv, dtype=np.float32)
    w_proj = np.ascontiguousarray(np.asarray(w_proj, dtype=np.float32))
    b_proj = np.asarray(b_proj, dtype=np.float32)

    wqk_bf = w_qkv[:, : 2 * D].astype(ml_dtypes.bfloat16)
    wv_bf = np.ascontiguousarray(w_qkv[:, 2 * D:]).astype(ml_dtypes.bfloat16)
    wp_bf = w_proj.astype(ml_dtypes.bfloat16)
    bqk_col = b_qkv[: 2 * D].reshape(2 * D, 1).copy()
    # v-bias folded through the output projection: probs rows sum to 1
    b_eff = (b_qkv[2 * D:] @ w_proj + b_proj).reshape(D, 1).astype(np.float32)

    hs_bf = hidden_states.astype(ml_dtypes.bfloat16)
    in_maps = []
    for c in range(N_CORES):
        xb = np.zeros((B, D, SPAD), dtype=ml_dtypes.bfloat16)
        xb[:, :, :S] = hs_bf[c * B:(c + 1) * B].transpose(0, 2, 1)
        in_maps.append({
            "xT_bf": xb,
            "wqk_bf": wqk_bf,
            "wv_bf": wv_bf,
            "wp_bf": wp_bf,
            "b_qk_col": bqk_col,
            "b_eff_col": b_eff,
        })
    return in_maps


def kernel(hidden_states, w_qkv, b_qkv, w_proj, b_proj):
    nc = _get_nc()
    in_maps = make_in_maps(hidden_states, w_qkv, b_qkv, w_proj, b_proj)
    res = run_bass_kernel_spmd(nc, in_maps, list(range(N_CORES)))
    outs = []
    for c in range(N_CORES):
        oT = np.asarray(res.results[c]["outT"], dtype=np.float32)  # [D, T]
        outs.append(oT.T.reshape(B, S, D))
    return np.concatenate(outs, axis=0).astype(np.float32)


if __name__ == "__main__":
    rng = np.random.default_rng(0)
    hs = rng.standard_normal((B_TOTAL, S, D), dtype=np.float32)
    wq = rng.standard_normal((D, 3 * D), dtype=np.float32) * D ** -0.5
    bq = rng.standard_normal(3 * D).astype(np.float32) * 0.02
    wp = rng.standard_normal((D, D), dtype=np.float32) * D ** -0.5
    bp = rng.standard_normal(D).astype(np.float32) * 0.02
    o = kernel(hidden_states=hs, w_qkv=wq, b_qkv=bq, w_proj=wp, b_proj=bp)
    print(o.shape, o.dtype)



# revision 7
# speedup vs baseline: 1.3040x; 1.3040x over previous
"""BlipAttention kernel for 8 Trainium2 NeuronCores.

Strategy: data-parallel over batch (16 batches -> 2 per core), no collectives.
Per core: fused QKV projection + 16-head scaled-dot-product attention + output
projection on the PE, bf16 matmuls with fp32 PSUM accumulation.

v3 restructure (from trace analysis of the v2 kernel; v2 spent 121us in DVE
RECIPROCAL on [1,578] single-partition rows, 35us in gpsimd
PartitionBroadcast, and stalled the PE during attention because pv PSUM tiles
were held ~5us per head through that slow normalization chain):
  - softmax normalization is deferred and batched: each head's raw PV output
    (88 v-rows + 1 fused-denominator row) is copied PSUM->SBUF bf16 in ~0.5us
    (releasing PSUM immediately), the raw v rows are DMA'd into the
    feature-major apk tiles unnormalized, and the denominator row is DMA'd
    into a per-batch [16, 578] den tile.
  - per batch, one DVE reciprocal_approx_fast over [16,578] replaces 32
    lane-starved [1,578] reciprocals; a 0/1 head-indicator matmul (stationary
    [16,128] per apk k-tile, moving f32r reciprocal rows) broadcasts the
    per-(head, token) scale into [128,578] PSUM tiles; one DVE multiply per
    apk tile applies it in place.  partition_broadcast is gone.
  - attention heads are emitted interleaved with the q/k projection m-tiles
    (paired q/k column order) so exp/PV work overlaps projection matmuls
    from the first head onward; batch-1 projections and batch-0 output
    projection interleave as before.
  - input loads are spread across four engine DMA queues.
  - vsb ones-columns shrink from 9 to 1 (VG 97->89), and only the ones
    column is memset.
"""

import contextlib

import numpy as np
import ml_dtypes

import concourse.bass as bass
import concourse.tile as tile
from concourse import bacc, mybir
from concourse.bass_utils import run_bass_kernel_spmd

F32 = mybir.dt.float32
F32R = mybir.dt.float32r
BF16 = mybir.dt.bfloat16

N_CORES = 8
B_TOTAL, S, D = 16, 577, 1408
H, HD = 16, 88
SCALE = HD ** -0.5
B = B_TOTAL // N_CORES          # batches per core = 2
T = B * S                       # tokens per core = 1154
SP = S + 1                      # padded q-span = 578
SPAD = 592                      # host-padded token rows per batch (37*16)
KT = D // 128                   # 11 k-tiles over D
VG = HD + 1                     # v group width per head: 88 v cols + 1 one
DEN = HD                        # psum partition of the softmax denominator

# chunks of a 578-wide span, each within one PSUM bank
CH_S = [(0, 512), (512, 66)]
# v projection chunk-groups: (src col base, src width, subs) where each sub
# is (moving col, psum col, width, first head); subs are head-aligned
# (multiple of 88) and live in one PSUM bank of the 2-bank acc tile
VCH = [
    (0, 880, [(0, 0, 440, 0), (440, 512, 440, 5)]),
    (880, 528, [(0, 0, 440, 10), (440, 512, 88, 15)]),
]


def _tok_tiles():
    out = []
    for tt in range((S + 127) // 128):
        t0 = tt * 128
        out.append((tt, t0, min(128, S - t0)))
    return out


TT = len(_tok_tiles())          # 5


def build_program():
    nc = bacc.Bacc("TRN2", target_bir_lowering=False, debug=False,
                   num_devices=N_CORES)

    x_ap = nc.dram_tensor("xT_bf", [B, D, SPAD], BF16, kind="ExternalInput").ap()
    wqk_ap = nc.dram_tensor("wqk_bf", [D, 2 * D], BF16, kind="ExternalInput").ap()
    wv_ap = nc.dram_tensor("wv_bf", [D, D], BF16, kind="ExternalInput").ap()
    wp_ap = nc.dram_tensor("wp_bf", [D, D], BF16, kind="ExternalInput").ap()
    bqk_ap = nc.dram_tensor("b_qk_col", [2 * D, 1], F32, kind="ExternalInput").ap()
    beff_ap = nc.dram_tensor("b_eff_col", [D, 1], F32, kind="ExternalInput").ap()
    eh_ap = nc.dram_tensor("ehead_f", [H, KT * 128], F32R,
                           kind="ExternalInput").ap()
    outT_ap = nc.dram_tensor("outT", [D, T], BF16, kind="ExternalOutput").ap()

    with tile.TileContext(nc) as tc, contextlib.ExitStack() as ctx:
        p_xT = ctx.enter_context(tc.tile_pool(name="xT", bufs=2 * KT))
        p_vsb = ctx.enter_context(tc.tile_pool(name="vsb", bufs=2 * TT))
        p_qk = ctx.enter_context(tc.tile_pool(name="qk", bufs=36))
        p_expT = ctx.enter_context(tc.tile_pool(name="expT", bufs=8))
        p_atr = ctx.enter_context(tc.tile_pool(name="atr", bufs=4))
        p_apk = ctx.enter_context(tc.tile_pool(name="apk", bufs=2 * KT))
        p_qksb = ctx.enter_context(tc.tile_pool(name="qksb", bufs=4))
        p_den = ctx.enter_context(tc.tile_pool(name="den", bufs=2))
        p_nrm = ctx.enter_context(tc.tile_pool(name="nrm", bufs=4))
        p_esb = ctx.enter_context(tc.tile_pool(name="esb", bufs=1))
        p_wqk = ctx.enter_context(tc.tile_pool(name="wqk", bufs=4))
        p_wp = ctx.enter_context(tc.tile_pool(name="wp", bufs=4))
        p_wv = ctx.enter_context(tc.tile_pool(name="wv", bufs=12))
        p_bias = ctx.enter_context(tc.tile_pool(name="bias", bufs=6))
        p_ot = ctx.enter_context(tc.tile_pool(name="ot", bufs=4))

        p_sc = ctx.enter_context(tc.tile_pool(name="psc", bufs=2, space="PSUM"))
        p_pv = ctx.enter_context(tc.tile_pool(name="ppv", bufs=2, space="PSUM"))
        p_acc = ctx.enter_context(tc.tile_pool(name="pacc", bufs=2, space="PSUM"))

        # x^T tiles, per batch (host pre-transposed to feature-major)
        xT = [[p_xT.tile([128, SPAD], BF16, tag="xT", name=f"xT{b}_{k}")
               for k in range(KT)] for b in range(B)]
        # v tiles per batch; col 88 of each 89-group memset to 1.0 (fused
        # softmax denominator), v-copies fill cols 0..87
        vsb = [[p_vsb.tile([128, H * VG], BF16, tag="vsb", name=f"vsb{b}_{tt}")
                for tt in range(TT)] for b in range(B)]
        apk = [[p_apk.tile([128, SP], BF16, tag="apk", name=f"apk{b}_{k}")
                for k in range(KT)] for b in range(B)]
        den16 = [p_den.tile([H, SP], BF16, tag="den", name=f"den{b}")
                 for b in range(B)]
        esb = p_esb.tile([H, KT * 128], F32R, tag="esb", name="esb")
        qh = [[None] * H for _ in range(B)]
        kh = [[None] * H for _ in range(B)]

        def emit_warmup():
            # ~5us of back-to-back dummy matmuls so the HAM clock gate is
            # released (K=8/8) by the time real operands arrive
            wsrc = p_qksb.tile([128, SP], BF16, tag="qksb", name="warm_src")
            nc.gpsimd.memset(wsrc[:], 0.0)
            wps = p_acc.tile([128, 512], F32, tag="acc", name="warm_ps")
            for _ in range(56):
                nc.tensor.matmul(wps[:, 0:128], wsrc[:, 0:128],
                                 wsrc[:, 0:128], start=True, stop=True)

        LOAD_ENGS = None

        def emit_loads(b):
            engs = [nc.sync, nc.scalar, nc.gpsimd]
            for k in range(KT):
                engs[k % 3].dma_start(xT[b][k][:],
                                      x_ap[b, k * 128:(k + 1) * 128, :])
            for tt in range(TT):
                ones = vsb[b][tt][:].rearrange("p (h g) -> p h g",
                                               g=VG)[:, :, DEN:DEN + 1]
                nc.gpsimd.memset(ones, 1.0)

        def emit_vproj(b):
            """one unit per (chunk-group, token-tile)"""
            for cg, (s0, sw, subs) in enumerate(VCH):
                wvt = []
                for k in range(KT):
                    wv_t = p_wv.tile([128, 880], BF16, tag="wv",
                                     name=f"wv{b}_{cg}_{k}")
                    eng = nc.gpsimd if k % 2 == 0 else nc.sync
                    eng.dma_start(
                        wv_t[:, 0:sw],
                        wv_ap[k * 128:(k + 1) * 128, s0:s0 + sw])
                    wvt.append(wv_t)
                for tt, t0, ts in _tok_tiles():
                    acc = p_acc.tile([128, 952], F32, tag="acc", name="vacc")
                    for ki in range(KT):
                        for (m0, p0, w, h0) in subs:
                            nc.tensor.matmul(
                                acc[0:ts, p0:p0 + w],
                                xT[b][ki][:, t0:t0 + ts],
                                wvt[ki][:, m0:m0 + w],
                                start=(ki == 0), stop=(ki == KT - 1))
                    for (m0, p0, w, h0) in subs:
                        nh = w // HD
                        dst = vsb[b][tt][0:ts, :].rearrange(
                            "p (h g) -> p h g", g=VG)[:, h0:h0 + nh, 0:HD]
                        src = acc[0:ts, p0:p0 + w].rearrange(
                            "p (h g) -> p h g", g=HD)
                        nc.vector.tensor_copy(dst, src)
                    yield

        def _ship_heads(b, which, fidx, qksb):
            dst_l = qh[b] if which == 0 else kh[b]
            f_lo, f_hi = fidx * 128, fidx * 128 + 128
            for h in range(f_lo // HD, min(H, (f_hi + HD - 1) // HD)):
                s0 = max(f_lo, h * HD)
                s1 = min(f_hi, (h + 1) * HD)
                if s1 <= s0:
                    continue
                if dst_l[h] is None:
                    dst_l[h] = p_qk.tile([HD, SP], BF16, tag="qk",
                                         name=f"qk{b}_{which}_{h}")
                r0 = s0 - h * HD
                nc.scalar.dma_start(dst_l[h][r0: r0 + (s1 - s0), :],
                                    qksb[s0 - f_lo: s1 - f_lo, :])

        def emit_qk_unit(b, m):
            """one q/k projection m-tile (m in 0..21; 0..10 q, 11..21 k)"""
            col = m * 128
            wqt = p_wqk.tile([128, KT * 128], BF16, tag="wqk",
                             name=f"wq{b}_{m}")
            nc.sync.dma_start(
                wqt[:].rearrange("p (k c) -> p k c", k=KT),
                wqk_ap[:, col: col + 128].rearrange("(k p) c -> p k c",
                                                    p=128))
            bq = p_bias.tile([128, 1], F32, tag="bias", name="bq")
            nc.sync.dma_start(bq[:], bqk_ap[col: col + 128, :])
            acc = p_acc.tile([128, SP], F32, tag="acc", name="qkacc")
            for ki in range(KT):
                for (lc, w) in CH_S:
                    nc.tensor.matmul(acc[0:128, lc:lc + w],
                                     wqt[:, ki * 128:(ki + 1) * 128],
                                     xT[b][ki][:, lc:lc + w],
                                     start=(ki == 0), stop=(ki == KT - 1))
            qksb = p_qksb.tile([128, SP], BF16, tag="qksb")
            nc.vector.tensor_scalar_add(qksb[:], acc[0:128, 0:SP], bq[:])
            which, fidx = (0, m) if m < KT else (1, m - KT)
            _ship_heads(b, which, fidx, qksb)

        def emit_att_head(b, h):
            """scores/exp/PV for one head; raw PV rows + denominator row are
            copied out bf16 immediately so the pv PSUM tiles free fast"""
            pv = [p_pv.tile([VG, 512], F32, tag="pv", name=f"pv{si}")
                  for si in range(2)]
            for tt, t0, ts in _tok_tiles():
                for si, (lc, w) in enumerate(CH_S):
                    sc = p_sc.tile([128, 512], F32, tag="sc", name="sc")
                    nc.tensor.matmul(sc[0:ts, 0:w],
                                     kh[b][h][:, t0:t0 + ts],
                                     qh[b][h][:, lc:lc + w],
                                     start=True, stop=True)
                    et = p_expT.tile([128, 512], BF16, tag="expT")
                    nc.scalar.activation(et[0:ts, 0:w], sc[0:ts, 0:w],
                                         mybir.ActivationFunctionType.Exp,
                                         scale=SCALE)
                    nc.tensor.matmul(pv[si][0:VG, 0:w],
                                     vsb[b][tt][0:ts, h * VG:(h + 1) * VG],
                                     et[0:ts, 0:w],
                                     start=(tt == 0), stop=(tt == TT - 1))
            atr = p_atr.tile([VG, SP], BF16, tag="atr", name="atr")
            nc.vector.tensor_copy(atr[0:VG, 0:512], pv[0][0:VG, 0:512])
            nc.vector.tensor_copy(atr[0:VG, 512:SP], pv[1][0:VG, 0:66])
            # raw (unnormalized) v rows into the feature-major apk tiles
            f0 = h * HD
            k0, r0 = f0 // 128, f0 % 128
            n0 = min(HD, 128 - r0)
            nc.scalar.dma_start(apk[b][k0][r0: r0 + n0, :], atr[0:n0, :])
            if n0 < HD:
                nc.scalar.dma_start(apk[b][k0 + 1][0: HD - n0, :],
                                    atr[n0:HD, :])
            # denominator row into the batch den tile
            nc.sync.dma_start(den16[b][h:h + 1, :], atr[DEN:DEN + 1, :])

        def emit_norm(b):
            """batched softmax normalization for one batch: reciprocal of the
            16 denominator rows, head-indicator matmul broadcast, one in-place
            DVE multiply per apk k-tile"""
            d16f = p_nrm.tile([H, SP], F32, tag="nrm", name="d16f")
            nc.vector.tensor_copy(d16f[:], den16[b][:])
            r16f = p_nrm.tile([H, SP], F32, tag="nrm", name="r16f")
            nc.vector.reciprocal_approx_fast(r16f[:], d16f[:])
            r16 = p_nrm.tile([H, SP], F32R, tag="nrm", name="r16")
            with nc.allow_low_precision(reason="softmax reciprocal rows"):
                nc.vector.tensor_copy(r16[:], r16f[:])
            r16r = r16[:]
            for k in range(KT):
                recb = p_acc.tile([128, SP], F32, tag="acc", name="recb")
                for (lc, w) in CH_S:
                    nc.tensor.matmul(recb[0:128, lc:lc + w],
                                     esb[:, k * 128:(k + 1) * 128],
                                     r16r[:, lc:lc + w],
                                     start=True, stop=True)
                nc.vector.tensor_mul(apk[b][k][:], apk[b][k][:],
                                     recb[0:128, 0:SP])

        def emit_op_unit(b, oc):
            """one output-projection feature tile"""
            wpt = p_wp.tile([128, KT * 128], BF16, tag="wp",
                            name=f"wp{b}_{oc}")
            nc.sync.dma_start(
                wpt[:].rearrange("p (k c) -> p k c", k=KT),
                wp_ap[:, oc * 128:(oc + 1) * 128].rearrange(
                    "(k p) c -> p k c", p=128))
            be = p_bias.tile([128, 1], F32, tag="bias", name="be")
            nc.sync.dma_start(be[:], beff_ap[oc * 128:(oc + 1) * 128, :])
            acc = p_acc.tile([128, SP], F32, tag="acc", name="oacc")
            for ki in range(KT):
                for (lc, w) in CH_S:
                    nc.tensor.matmul(acc[0:128, lc:lc + w],
                                     wpt[:, ki * 128:(ki + 1) * 128],
                                     apk[b][ki][:, lc:lc + w],
                                     start=(ki == 0), stop=(ki == KT - 1))
            ot = p_ot.tile([128, SP], BF16, tag="ot")
            nc.vector.tensor_scalar_add(ot[:], acc[0:128, 0:SP], be[:])
            nc.sync.dma_start(outT_ap[oc * 128:(oc + 1) * 128,
                                      b * S:(b + 1) * S],
                              ot[:, 0:S])

        # ---- emission schedule ----
        # q/k m-tiles are emitted in (q_j, k_j) pairs; head h's attention is
        # emitted as soon as pairs 0..j cover its features, so exp/PV overlap
        # the remaining projection matmuls.
        def head_ready(j):
            return min(H, ((j + 1) * 128) // HD)

        emit_warmup()
        nc.sync.dma_start(esb[:], eh_ap[:, :])
        emit_loads(0)
        emit_loads(1)

        for _ in emit_vproj(0):
            pass
        emitted = 0
        for j in range(KT):
            emit_qk_unit(0, j)
            emit_qk_unit(0, KT + j)
            while emitted < head_ready(j):
                emit_att_head(0, emitted)
                emitted += 1

        for _ in emit_vproj(1):
            pass
        emit_norm(0)

        emitted = 0
        for j in range(KT):
            emit_qk_unit(1, j)
            emit_qk_unit(1, KT + j)
            while emitted < head_ready(j):
                emit_att_head(1, emitted)
                emitted += 1
            emit_op_unit(0, j)

        emit_norm(1)
        for oc in range(KT):
            emit_op_unit(1, oc)

    nc.compile()
    return nc


_NC_CACHE = None


def _get_nc():
    global _NC_CACHE
    if _NC_CACHE is None:
        _NC_CACHE = build_program()
    return _NC_CACHE


def make_in_maps(hidden_states, w_qkv, b_qkv, w_proj, b_proj):
    hidden_states = np.asarray(hidden_states, dtype=np.float32)
    w_qkv = np.ascontiguousarray(np.asarray(w_qkv, dtype=np.float32))
    b_qkv = np.asarray(b_qkv, dtype=np.float32)
    w_proj = np.ascontiguousarray(np.asarray(w_proj, dtype=np.float32))
    b_proj = np.asarray(b_proj, dtype=np.float32)

    wqk_bf = w_qkv[:, : 2 * D].astype(ml_dtypes.bfloat16)
    wv_bf = np.ascontiguousarray(w_qkv[:, 2 * D:]).astype(ml_dtypes.bfloat16)
    wp_bf = w_proj.astype(ml_dtypes.bfloat16)
    bqk_col = b_qkv[: 2 * D].reshape(2 * D, 1).copy()
    # v-bias folded through the output projection: probs rows sum to 1
    b_eff = (b_qkv[2 * D:] @ w_proj + b_proj).reshape(D, 1).astype(np.float32)

    # head-indicator matrix: ehead[h, k*128 + p] = 1 iff feature 128k+p
    # belongs to head h
    feat = np.arange(KT * 128)
    ehead = (feat[None, :] // HD == np.arange(H)[:, None])
    ehead_f = ehead.astype(np.float32)

    hs_bf = hidden_states.astype(ml_dtypes.bfloat16)
    in_maps = []
    for c in range(N_CORES):
        xb = np.zeros((B, D, SPAD), dtype=ml_dtypes.bfloat16)
        xb[:, :, :S] = hs_bf[c * B:(c + 1) * B].transpose(0, 2, 1)
        in_maps.append({
            "xT_bf": xb,
            "wqk_bf": wqk_bf,
            "wv_bf": wv_bf,
            "wp_bf": wp_bf,
            "b_qk_col": bqk_col,
            "b_eff_col": b_eff,
            "ehead_f": ehead_f,
        })
    return in_maps


def kernel(hidden_states, w_qkv, b_qkv, w_proj, b_proj):
    nc = _get_nc()
    in_maps = make_in_maps(hidden_states, w_qkv, b_qkv, w_proj, b_proj)
    res = run_bass_kernel_spmd(nc, in_maps, list(range(N_CORES)))
    outs = []
    for c in range(N_CORES):
        oT = np.asarray(res.results[c]["outT"], dtype=np.float32)  # [D, T]
        outs.append(oT.T.reshape(B, S, D))
    return np.concatenate(outs, axis=0).astype(np.float32)


if __name__ == "__main__":
    rng = np.random.default_rng(0)
    hs = rng.standard_normal((B_TOTAL, S, D), dtype=np.float32)
    wq = rng.standard_normal((D, 3 * D), dtype=np.float32) * D ** -0.5
    bq = rng.standard_normal(3 * D).astype(np.float32) * 0.02
    wp = rng.standard_normal((D, D), dtype=np.float32) * D ** -0.5
    bp = rng.standard_normal(D).astype(np.float32) * 0.02
    o = kernel(hidden_states=hs, w_qkv=wq, b_qkv=bq, w_proj=wp, b_proj=bp)
    print(o.shape, o.dtype)


# revision 11
# speedup vs baseline: 1.3481x; 1.0338x over previous
"""BlipAttention kernel for 8 Trainium2 NeuronCores.

Strategy: data-parallel over batch (16 batches -> 2 per core), no collectives.
Per core: fused QKV projection + 16-head scaled-dot-product attention + output
projection on the PE, bf16 matmuls with fp32 PSUM accumulation.

v3 restructure (from trace analysis of the v2 kernel; v2 spent 121us in DVE
RECIPROCAL on [1,578] single-partition rows, 35us in gpsimd
PartitionBroadcast, and stalled the PE during attention because pv PSUM tiles
were held ~5us per head through that slow normalization chain):
  - softmax normalization is deferred and batched: each head's raw PV output
    (88 v-rows + 1 fused-denominator row) is copied PSUM->SBUF bf16 in ~0.5us
    (releasing PSUM immediately), the raw v rows are DMA'd into the
    feature-major apk tiles unnormalized, and the denominator row is DMA'd
    into a per-batch [16, 578] den tile.
  - per batch, one DVE reciprocal_approx_fast over [16,578] replaces 32
    lane-starved [1,578] reciprocals; a 0/1 head-indicator matmul (stationary
    [16,128] per apk k-tile, moving f32r reciprocal rows) broadcasts the
    per-(head, token) scale into [128,578] PSUM tiles; one DVE multiply per
    apk tile applies it in place.  partition_broadcast is gone.
  - attention heads are emitted interleaved with the q/k projection m-tiles
    (paired q/k column order) so exp/PV work overlaps projection matmuls
    from the first head onward; batch-1 projections and batch-0 output
    projection interleave as before.
  - input loads are spread across four engine DMA queues.
  - vsb ones-columns shrink from 9 to 1 (VG 97->89), and only the ones
    column is memset.
"""

import contextlib

import numpy as np
import ml_dtypes

import concourse.bass as bass
import concourse.tile as tile
from concourse import bacc, mybir
from concourse.bass_utils import run_bass_kernel_spmd

F32 = mybir.dt.float32
F32R = mybir.dt.float32r
BF16 = mybir.dt.bfloat16

N_CORES = 8
B_TOTAL, S, D = 16, 577, 1408
H, HD = 16, 88
SCALE = HD ** -0.5
B = B_TOTAL // N_CORES          # batches per core = 2
T = B * S                       # tokens per core = 1154
SP = S + 1                      # padded q-span = 578
SPAD = 592                      # host-padded token rows per batch (37*16)
KT = D // 128                   # 11 k-tiles over D
VG = HD + 1                     # v group width per head: 88 v cols + 1 one
DEN = HD                        # psum partition of the softmax denominator

# chunks of a 578-wide span, each within one PSUM bank
CH_S = [(0, 512), (512, 66)]
# v projection chunk-groups: (src col base, src width, subs) where each sub
# is (moving col, psum col, width, first head); subs are head-aligned
# (multiple of 88) and live in one PSUM bank of the 2-bank acc tile
VCH = [
    (0, 880, [(0, 0, 440, 0), (440, 512, 440, 5)]),
    (880, 528, [(0, 0, 440, 10), (440, 512, 88, 15)]),
]


def _tok_tiles():
    out = []
    for tt in range((S + 127) // 128):
        t0 = tt * 128
        out.append((tt, t0, min(128, S - t0)))
    return out


TT = len(_tok_tiles())          # 5


def build_program():
    nc = bacc.Bacc("TRN2", target_bir_lowering=False, debug=False,
                   num_devices=N_CORES)

    x_ap = nc.dram_tensor("xT_bf", [B, D, SPAD], BF16, kind="ExternalInput").ap()
    wqk_ap = nc.dram_tensor("wqk_bf", [D, 2 * D], BF16, kind="ExternalInput").ap()
    wv_ap = nc.dram_tensor("wv_bf", [D, D], BF16, kind="ExternalInput").ap()
    wp_ap = nc.dram_tensor("wp_bf", [D, D], BF16, kind="ExternalInput").ap()
    bqk_ap = nc.dram_tensor("b_qk_col", [2 * D, 1], F32, kind="ExternalInput").ap()
    beff_ap = nc.dram_tensor("b_eff_col", [D, 1], F32, kind="ExternalInput").ap()
    eh_ap = nc.dram_tensor("ehead_bf", [H, KT * 128], BF16,
                           kind="ExternalInput").ap()
    outT_ap = nc.dram_tensor("outT", [D, T], BF16, kind="ExternalOutput").ap()

    with tile.TileContext(nc) as tc, contextlib.ExitStack() as ctx:
        p_xT = ctx.enter_context(tc.tile_pool(name="xT", bufs=2 * KT))
        p_vsb = ctx.enter_context(tc.tile_pool(name="vsb", bufs=2 * TT))
        p_qk = ctx.enter_context(tc.tile_pool(name="qk", bufs=32))
        p_expT = ctx.enter_context(tc.tile_pool(name="expT", bufs=6))
        p_atr = ctx.enter_context(tc.tile_pool(name="atr", bufs=4))
        p_apk = ctx.enter_context(tc.tile_pool(name="apk", bufs=2 * KT))
        p_qksb = ctx.enter_context(tc.tile_pool(name="qksb", bufs=4))
        p_den = ctx.enter_context(tc.tile_pool(name="den", bufs=2))
        p_nrm = ctx.enter_context(tc.tile_pool(name="nrm", bufs=3))
        p_esb = ctx.enter_context(tc.tile_pool(name="esb", bufs=1))
        p_wqk = ctx.enter_context(tc.tile_pool(name="wqk", bufs=4))
        p_wp = ctx.enter_context(tc.tile_pool(name="wp", bufs=4))
        p_wv = ctx.enter_context(tc.tile_pool(name="wv", bufs=22))
        p_bias = ctx.enter_context(tc.tile_pool(name="bias", bufs=6))
        p_ot = ctx.enter_context(tc.tile_pool(name="ot", bufs=4))

        p_sc = ctx.enter_context(tc.tile_pool(name="psc", bufs=2, space="PSUM"))
        p_pv = ctx.enter_context(tc.tile_pool(name="ppv", bufs=2, space="PSUM"))
        p_acc = ctx.enter_context(tc.tile_pool(name="pacc", bufs=2, space="PSUM"))

        # x^T tiles, per batch (host pre-transposed to feature-major)
        xT = [[p_xT.tile([128, SPAD], BF16, tag="xT", name=f"xT{b}_{k}")
               for k in range(KT)] for b in range(B)]
        # v tiles per batch; col 88 of each 89-group memset to 1.0 (fused
        # softmax denominator), v-copies fill cols 0..87
        vsb = [[p_vsb.tile([128, H * VG], BF16, tag="vsb", name=f"vsb{b}_{tt}")
                for tt in range(TT)] for b in range(B)]
        apk = [[p_apk.tile([128, SP], BF16, tag="apk", name=f"apk{b}_{k}")
                for k in range(KT)] for b in range(B)]
        den16 = [p_den.tile([H, SP], BF16, tag="den", name=f"den{b}")
                 for b in range(B)]
        esb = p_esb.tile([H, KT * 128], BF16, tag="esb", name="esb")
        qh = [[None] * H for _ in range(B)]
        kh = [[None] * H for _ in range(B)]

        def emit_warmup():
            # ~5us of back-to-back dummy matmuls so the HAM clock gate is
            # released (K=8/8) by the time real operands arrive
            wsrc = p_qksb.tile([128, SP], BF16, tag="qksb", name="warm_src")
            nc.gpsimd.memset(wsrc[:], 0.0)
            wps = p_acc.tile([128, 512], F32, tag="acc", name="warm_ps")
            for _ in range(56):
                nc.tensor.matmul(wps[:, 0:128], wsrc[:, 0:128],
                                 wsrc[:, 0:128], start=True, stop=True)

        LOAD_ENGS = None

        def emit_loads(b):
            engs = [nc.sync, nc.scalar, nc.gpsimd]
            for k in range(KT):
                engs[k % 3].dma_start(xT[b][k][:],
                                      x_ap[b, k * 128:(k + 1) * 128, :])
            for tt in range(TT):
                ones = vsb[b][tt][:].rearrange("p (h g) -> p h g",
                                               g=VG)[:, :, DEN:DEN + 1]
                nc.gpsimd.memset(ones, 1.0)

        def prefetch_wv(b):
            wvt = [[None] * KT for _ in VCH]
            for cg, (s0, sw, subs) in enumerate(VCH):
                for k in range(KT):
                    wv_t = p_wv.tile([128, 880], BF16, tag="wv",
                                     name=f"wv{b}_{cg}_{k}")
                    nc.gpsimd.dma_start(
                        wv_t[:, 0:sw],
                        wv_ap[k * 128:(k + 1) * 128, s0:s0 + sw])
                    wvt[cg][k] = wv_t
            return wvt

        def emit_vproj(b, wvt_all):
            """one unit per (chunk-group, token-tile)"""
            for cg, (s0, sw, subs) in enumerate(VCH):
                wvt = wvt_all[cg]
                for tt, t0, ts in _tok_tiles():
                    acc = p_acc.tile([128, 952], F32, tag="acc", name="vacc")
                    for ki in range(KT):
                        for (m0, p0, w, h0) in subs:
                            nc.tensor.matmul(
                                acc[0:ts, p0:p0 + w],
                                xT[b][ki][:, t0:t0 + ts],
                                wvt[ki][:, m0:m0 + w],
                                start=(ki == 0), stop=(ki == KT - 1))
                    for (m0, p0, w, h0) in subs:
                        nh = w // HD
                        dst = vsb[b][tt][0:ts, :].rearrange(
                            "p (h g) -> p h g", g=VG)[:, h0:h0 + nh, 0:HD]
                        src = acc[0:ts, p0:p0 + w].rearrange(
                            "p (h g) -> p h g", g=HD)
                        nc.vector.tensor_copy(dst, src)
                    yield

        def _ship_heads(b, which, fidx, qksb):
            dst_l = qh[b] if which == 0 else kh[b]
            f_lo, f_hi = fidx * 128, fidx * 128 + 128
            for h in range(f_lo // HD, min(H, (f_hi + HD - 1) // HD)):
                s0 = max(f_lo, h * HD)
                s1 = min(f_hi, (h + 1) * HD)
                if s1 <= s0:
                    continue
                if dst_l[h] is None:
                    dst_l[h] = p_qk.tile([HD, SP], BF16, tag="qk",
                                         name=f"qk{b}_{which}_{h}")
                r0 = s0 - h * HD
                nc.scalar.dma_start(dst_l[h][r0: r0 + (s1 - s0), :],
                                    qksb[s0 - f_lo: s1 - f_lo, :])

        def emit_qk_unit(b, m):
            """one q/k projection m-tile (m in 0..21; 0..10 q, 11..21 k)"""
            col = m * 128
            wqt = p_wqk.tile([128, KT * 128], BF16, tag="wqk",
                             name=f"wq{b}_{m}")
            nc.sync.dma_start(
                wqt[:].rearrange("p (k c) -> p k c", k=KT),
                wqk_ap[:, col: col + 128].rearrange("(k p) c -> p k c",
                                                    p=128))
            bq = p_bias.tile([128, 1], F32, tag="bias", name="bq")
            nc.sync.dma_start(bq[:], bqk_ap[col: col + 128, :])
            acc = p_acc.tile([128, SP], F32, tag="acc", name="qkacc")
            for ki in range(KT):
                for (lc, w) in CH_S:
                    nc.tensor.matmul(acc[0:128, lc:lc + w],
                                     wqt[:, ki * 128:(ki + 1) * 128],
                                     xT[b][ki][:, lc:lc + w],
                                     start=(ki == 0), stop=(ki == KT - 1))
            qksb = p_qksb.tile([128, SP], BF16, tag="qksb")
            nc.vector.tensor_scalar_add(qksb[:], acc[0:128, 0:SP], bq[:])
            which, fidx = (0, m) if m < KT else (1, m - KT)
            _ship_heads(b, which, fidx, qksb)

        def emit_att_head(b, h):
            """scores/exp/PV for one head; raw PV rows + denominator row are
            copied out bf16 immediately so the pv PSUM tiles free fast"""
            pv = [p_pv.tile([VG, 512], F32, tag="pv", name=f"pv{si}")
                  for si in range(2)]
            for tt, t0, ts in _tok_tiles():
                for si, (lc, w) in enumerate(CH_S):
                    sc = p_sc.tile([128, 512], F32, tag="sc", name="sc")
                    nc.tensor.matmul(sc[0:ts, 0:w],
                                     kh[b][h][:, t0:t0 + ts],
                                     qh[b][h][:, lc:lc + w],
                                     start=True, stop=True)
                    et = p_expT.tile([128, 512], BF16, tag="expT")
                    nc.scalar.activation(et[0:ts, 0:w], sc[0:ts, 0:w],
                                         mybir.ActivationFunctionType.Exp,
                                         scale=SCALE)
                    nc.tensor.matmul(pv[si][0:VG, 0:w],
                                     vsb[b][tt][0:ts, h * VG:(h + 1) * VG],
                                     et[0:ts, 0:w],
                                     start=(tt == 0), stop=(tt == TT - 1))
            atr = p_atr.tile([VG, SP], BF16, tag="atr", name="atr")
            nc.vector.tensor_copy(atr[0:VG, 0:512], pv[0][0:VG, 0:512])
            nc.vector.tensor_copy(atr[0:VG, 512:SP], pv[1][0:VG, 0:66])
            # raw (unnormalized) v rows into the feature-major apk tiles
            f0 = h * HD
            k0, r0 = f0 // 128, f0 % 128
            n0 = min(HD, 128 - r0)
            nc.scalar.dma_start(apk[b][k0][r0: r0 + n0, :], atr[0:n0, :])
            if n0 < HD:
                nc.scalar.dma_start(apk[b][k0 + 1][0: HD - n0, :],
                                    atr[n0:HD, :])
            # denominator row into the batch den tile
            nc.sync.dma_start(den16[b][h:h + 1, :], atr[DEN:DEN + 1, :])

        def emit_norm(b):
            """batched softmax normalization for one batch: reciprocal of the
            16 denominator rows, head-indicator matmul broadcast, one in-place
            DVE multiply per apk k-tile"""
            d16f = p_nrm.tile([H, SP], F32, tag="nrm", name="d16f")
            nc.vector.tensor_copy(d16f[:], den16[b][:])
            r16f = p_nrm.tile([H, SP], F32, tag="nrm", name="r16f")
            nc.vector.reciprocal_approx_fast(r16f[:], d16f[:])
            r16 = p_nrm.tile([H, SP], BF16, tag="nrm", name="r16")
            nc.vector.tensor_copy(r16[:], r16f[:])
            r16r = r16[:]
            for k in range(KT):
                recb = p_acc.tile([128, SP], F32, tag="acc", name="recb")
                for (lc, w) in CH_S:
                    nc.tensor.matmul(recb[0:128, lc:lc + w],
                                     esb[:, k * 128:(k + 1) * 128],
                                     r16r[:, lc:lc + w],
                                     start=True, stop=True)
                nc.vector.tensor_mul(apk[b][k][:], apk[b][k][:],
                                     recb[0:128, 0:SP])

        def emit_op_unit(b, oc):
            """one output-projection feature tile"""
            wpt = p_wp.tile([128, KT * 128], BF16, tag="wp",
                            name=f"wp{b}_{oc}")
            nc.sync.dma_start(
                wpt[:].rearrange("p (k c) -> p k c", k=KT),
                wp_ap[:, oc * 128:(oc + 1) * 128].rearrange(
                    "(k p) c -> p k c", p=128))
            be = p_bias.tile([128, 1], F32, tag="bias", name="be")
            nc.sync.dma_start(be[:], beff_ap[oc * 128:(oc + 1) * 128, :])
            acc = p_acc.tile([128, SP], F32, tag="acc", name="oacc")
            for ki in range(KT):
                for (lc, w) in CH_S:
                    nc.tensor.matmul(acc[0:128, lc:lc + w],
                                     wpt[:, ki * 128:(ki + 1) * 128],
                                     apk[b][ki][:, lc:lc + w],
                                     start=(ki == 0), stop=(ki == KT - 1))
            ot = p_ot.tile([128, SP], BF16, tag="ot")
            nc.vector.tensor_scalar_add(ot[:], acc[0:128, 0:SP], be[:])
            nc.sync.dma_start(outT_ap[oc * 128:(oc + 1) * 128,
                                      b * S:(b + 1) * S],
                              ot[:, 0:S])

        # ---- emission schedule ----
        # q/k m-tiles are emitted in (q_j, k_j) pairs; head h's attention is
        # emitted as soon as pairs 0..j cover its features, so exp/PV overlap
        # the remaining projection matmuls.
        def head_ready(j):
            return min(H, ((j + 1) * 128) // HD)

        emit_warmup()
        nc.sync.dma_start(esb[:], eh_ap[:, :])
        emit_loads(0)
        wvt0 = prefetch_wv(0)
        emit_loads(1)

        for _ in emit_vproj(0, wvt0):
            pass
        wvt1 = None
        emitted = 0
        for j in range(KT):
            emit_qk_unit(0, j)
            emit_qk_unit(0, KT + j)
            while emitted < head_ready(j):
                emit_att_head(0, emitted)
                emitted += 1
            if j == 2:
                wvt1 = prefetch_wv(1)

        for _ in emit_vproj(1, wvt1):
            pass
        emit_norm(0)

        emitted = 0
        for j in range(KT):
            emit_qk_unit(1, j)
            emit_qk_unit(1, KT + j)
            while emitted < head_ready(j):
                emit_att_head(1, emitted)
                emitted += 1
            emit_op_unit(0, j)

        emit_norm(1)
        for oc in range(KT):
            emit_op_unit(1, oc)

    nc.compile()
    return nc


_NC_CACHE = None


def _get_nc():
    global _NC_CACHE
    if _NC_CACHE is None:
        _NC_CACHE = build_program()
    return _NC_CACHE


def make_in_maps(hidden_states, w_qkv, b_qkv, w_proj, b_proj):
    hidden_states = np.asarray(hidden_states, dtype=np.float32)
    w_qkv = np.ascontiguousarray(np.asarray(w_qkv, dtype=np.float32))
    b_qkv = np.asarray(b_qkv, dtype=np.float32)
    w_proj = np.ascontiguousarray(np.asarray(w_proj, dtype=np.float32))
    b_proj = np.asarray(b_proj, dtype=np.float32)

    wqk_bf = w_qkv[:, : 2 * D].astype(ml_dtypes.bfloat16)
    wv_bf = np.ascontiguousarray(w_qkv[:, 2 * D:]).astype(ml_dtypes.bfloat16)
    wp_bf = w_proj.astype(ml_dtypes.bfloat16)
    bqk_col = b_qkv[: 2 * D].reshape(2 * D, 1).copy()
    # v-bias folded through the output projection: probs rows sum to 1
    b_eff = (b_qkv[2 * D:] @ w_proj + b_proj).reshape(D, 1).astype(np.float32)

    # head-indicator matrix: ehead[h, k*128 + p] = 1 iff feature 128k+p
    # belongs to head h
    feat = np.arange(KT * 128)
    ehead = (feat[None, :] // HD == np.arange(H)[:, None])
    ehead_bf = ehead.astype(ml_dtypes.bfloat16)

    hs_bf = hidden_states.astype(ml_dtypes.bfloat16)
    in_maps = []
    for c in range(N_CORES):
        xb = np.zeros((B, D, SPAD), dtype=ml_dtypes.bfloat16)
        xb[:, :, :S] = hs_bf[c * B:(c + 1) * B].transpose(0, 2, 1)
        in_maps.append({
            "xT_bf": xb,
            "wqk_bf": wqk_bf,
            "wv_bf": wv_bf,
            "wp_bf": wp_bf,
            "b_qk_col": bqk_col,
            "b_eff_col": b_eff,
            "ehead_bf": ehead_bf,
        })
    return in_maps


def kernel(hidden_states, w_qkv, b_qkv, w_proj, b_proj):
    nc = _get_nc()
    in_maps = make_in_maps(hidden_states, w_qkv, b_qkv, w_proj, b_proj)
    res = run_bass_kernel_spmd(nc, in_maps, list(range(N_CORES)))
    outs = []
    for c in range(N_CORES):
        oT = np.asarray(res.results[c]["outT"], dtype=np.float32)  # [D, T]
        outs.append(oT.T.reshape(B, S, D))
    return np.concatenate(outs, axis=0).astype(np.float32)


if __name__ == "__main__":
    rng = np.random.default_rng(0)
    hs = rng.standard_normal((B_TOTAL, S, D), dtype=np.float32)
    wq = rng.standard_normal((D, 3 * D), dtype=np.float32) * D ** -0.5
    bq = rng.standard_normal(3 * D).astype(np.float32) * 0.02
    wp = rng.standard_normal((D, D), dtype=np.float32) * D ** -0.5
    bp = rng.standard_normal(D).astype(np.float32) * 0.02
    o = kernel(hidden_states=hs, w_qkv=wq, b_qkv=bq, w_proj=wp, b_proj=bp)
    print(o.shape, o.dtype)
